# revision 12
# baseline (speedup 1.0000x reference)
"""SNN (soft-nearest-neighbor) contrastive loss on 8 Trainium2 NeuronCores.

Math
----
z = concat(x, y) in R^{8192x128};  d_ij = ||z_i - z_j||.
The row max subtracted in the reference cancels mathematically, so
    S0_i  = sum_{j != i} exp(-d_ij)          (device + host gather)
    dp_i  = d_{i, pair(i)}                   (host, O(N*D))
    loss  = mean_i( -log( exp(-dp_i)/S0_i + tiny ) )   (host, trivial)

Symmetry halving
----------------
d_ij is symmetric; each 128-row block R computes exp tiles for column
blocks R..R+32 only (self + 32 forward, cyclically).  Strip = 4224 cols.
Row sums cover the WHOLE strip (the antipodal block, offset 32, is
computed by both partners for their own rows).  Column sums cover
offsets 1..31 and are scattered on the host into the mirrored rows.

Device pipeline (one SPMD program, 8 cores, rows sharded 1024/core)
------------------------------------------------------------------
PE: fp8e4 DoubleRow matmuls with 65-row k-tiles compute
      Q = u.u^T - hsq_j      (u split into 2x64 dims; the 65th row of
k-tile 0/1 carries ones x -hsq_hi / ones x -hsq_lo, folding the hsq_j
rank-1 term into the main matmul), plus an FD-128 bf16 identity matmul
adding -LARGE on the self diagonal.
ACT: w = A16*sqrt(-Q + hsq_i) straight from PSUM via the per-partition
bias AP (bf16 out), one instruction per triple-buffered [128,1024]
PSUM tile.  ACT runs 1x/1.2GHz and is the ~40us critical engine; PE
and DVE work is split so both fit in its shadow.
DVE: Schraudolph bit trick -- one 4x-mode tensor_scalar produces
codes = int16(B16 - w) = the bf16 bit pattern of ~exp(-d)*SCALE_COMP.
Row sums: three 2x-mode fold adds then a 1x accumulate pass over the
last 528-col eighth.
Column sums, split to balance PE vs DVE:
  - prefold strips (0,4,5,6): 2x-mode adds (strip 0: 4x copy), aligned
    by rotated column, into one bf16 accumulator facc covering rotated
    cols [128, 4864); a single PE one-hot pass colsums facc at the end.
  - direct strips (1,2,3,7): PE one-hot colsum matmuls straight from
    the codes tile, emitted one strip late so the PE never stalls
    waiting on codes.
Each core gets column-ROTATED operands so every tile index is a
compile-time constant: one identical program for all 8 cores.
"""

import os
import sys
from contextlib import ExitStack

import numpy as np

_TRN_REPO = os.environ.get("TRN_RL_REPO", "/opt/trn_rl_repo")
if _TRN_REPO not in sys.path:
    sys.path.insert(0, _TRN_REPO)

import ml_dtypes

BF16 = ml_dtypes.bfloat16

B = 4096
D = 128
N = 2 * B            # 8192 rows of z
NCORES = 8
RPC = N // NCORES    # 1024 rows per core
S = RPC // 128       # 8 row-subtiles per core
CT = 512             # matmul moving tile (one PSUM bank = 512 f32)
SL = 4224            # strip length: self block + 32 forward blocks
CW = 4096            # colsum window end (blocks 1..31): [base+128, base+CW)
PT = 1024            # PSUM strip tile columns (2 banks)
UCOLS = 5120         # rotated cols touched: [0, 128*(S-1) + SL) = 5120
NCH = 10             # colsum chunks of 512 covering rotated cols [0, 5120)
FACC = (S - 2) * 128 + CW - 128   # 4736 facc cols = rotated [128, 4864)
LARGE = 7296.0       # diagonal nuke: d2 -> 7296, w -> A16*85.4 = 15772:
                     #   code = B16 - 15772 = 484 -> decoded ~4e-43 (= 0)

PREFOLD = (0, 1, 2, 3)   # strips folded into facc (0 is a copy)
WARMUP = 2

LN2 = float(np.log(2.0))
A16 = 128.0 / LN2    # bf16 exponent-code slope
B16 = 16256.0        # bf16 exponent-code offset (exact in bf16)
# with B16=16256 the Schraudolph decode averages exp(-w)*SCALE_COMP;
# the host divides all device sums by SCALE_COMP.
SCALE_COMP = 1.0406027025852233  # mean of (1+f)/2^f over f~U[0,1)

PROFILE = False
LAST_RESULT = None

_cache = {}


def _build_program():
    import concourse.tile as tile
    from concourse import bacc, mybir

    f32 = mybir.dt.float32
    bf16 = mybir.dt.bfloat16
    i16 = mybir.dt.int16
    fp8 = mybir.dt.float8e4
    AF = mybir.ActivationFunctionType
    OP = mybir.AluOpType
    PM = mybir.MatmulPerfMode

    nc = bacc.Bacc()

    u8 = mybir.dt.uint8
    h_u2 = nc.declare_dram_parameter("u2", [65, 2, UCOLS], fp8, isOutput=False)
    h_u2w = nc.declare_dram_parameter("u2w", [65, 2, S * 128], fp8, isOutput=False)
    # packed consts, one DMA: dmask | oneh | hsqpa (bytes)
    h_consts = nc.declare_dram_parameter("consts", [128, 488], u8, isOutput=False)
    h_s0 = nc.declare_dram_parameter("s0", [128, S], f32, isOutput=True)
    h_cs = nc.declare_dram_parameter("cs", [NCH, CT], f32, isOutput=True)

    # strip for subtile s covers rotated cols [s*128, s*128 + SL)
    with tile.TileContext(nc) as tc, ExitStack() as ctx:
        const = ctx.enter_context(tc.tile_pool(name="const", bufs=1))
        wpool = ctx.enter_context(tc.tile_pool(name="wbuf", bufs=3))
        cpool = ctx.enter_context(tc.tile_pool(name="codes", bufs=3))
        spool = ctx.enter_context(tc.tile_pool(name="scr", bufs=2))
        pspool = ctx.enter_context(tc.tile_pool(name="ps", bufs=3, space="PSUM"))
        pstail = ctx.enter_context(tc.tile_pool(name="pst", bufs=1, space="PSUM"))
        cspool = ctx.enter_context(tc.tile_pool(name="cps", bufs=1, space="PSUM"))
        misc = ctx.enter_context(tc.tile_pool(name="misc", bufs=1))

        # DMA priority: first matmul's operands first, then packed consts,
        # then the rest of u2 in chunks
        t_u2w = const.tile([65, 2, S * 128], fp8)
        nc.sync.dma_start(out=t_u2w[:], in_=h_u2w[:])
        t_u2 = const.tile([65, 2, UCOLS], fp8)
        edges = [0, 512, 1024, 2048, 3072, 4096, 5120]
        for a, b in zip(edges[:2], edges[1:3]):
            nc.sync.dma_start(out=t_u2[:, :, a:b], in_=h_u2[:, :, a:b])

        t_consts = const.tile([128, 488], u8)
        nc.sync.dma_start(out=t_consts[:], in_=h_consts[:])
        t_dmask = t_consts[:, 0:256].bitcast(i16)
        t_oneh = t_consts[:, 256:456].bitcast(bf16)
        t_hsqpa = t_consts[:, 456:488].bitcast(f32)

        for a, b in zip(edges[2:-1], edges[3:]):
            nc.sync.dma_start(out=t_u2[:, :, a:b], in_=h_u2[:, :, a:b])

        t_zero10 = const.tile([128, NCH], bf16)
        nc.vector.memset(t_zero10[:], 0.0)
        t_z512 = const.tile([128, CT], bf16)
        nc.vector.memset(t_z512[:], 0.0)

        s0_t = const.tile([128, S], f32)
        facc = const.tile([128, FACC], bf16)
        # zero the tail of facc not covered by the strip-0 copy
        nc.gpsimd.memset(facc[:, CW - 128:FACC], 0.0)

        # dummy tiny sqrt: triggers the sqrt ACT_TABLE_LOAD during the
        # input DMA instead of lazily before the first real sqrt
        t_dummy = const.tile([128, 1], f32)
        nc.vector.memset(t_dummy[:], 1.0)
        dummyout = const.tile([128, 1], bf16)
        nc.scalar.activation(
            out=dummyout[:], in_=t_dummy[:], func=AF.Sqrt, scale=1.0
        )

        # single resident colsum accumulator [NCH, 512]
        cs_acc = cspool.tile([NCH, CT], f32, tag="cs", name="cs_acc")

        # zero the colsum accumulator (matmul with zero weights); also
        # keeps the PE busy during the input DMA
        for rep in range(WARMUP):
            nc.tensor.matmul(
                cs_acc[:], t_zero10[:], t_z512[:],
                start=(rep == 0), stop=False, skip_group_check=True,
            )

        pending = []

        def colsum_mm(cs_a, cs_b, j, etile, e_a, e_b):
            nc.tensor.matmul(
                cs_acc[:, cs_a:cs_b],
                t_oneh[:, NCH * j:NCH * (j + 1)],
                etile[:, e_a:e_b],
                start=False,
                stop=False,
                skip_group_check=True,
            )

        def colsums(s, etile, rlo, rhi, defer=False):
            # accumulate per-column sums of etile cols [rlo, rhi) (strip-
            # local) into cs_acc at their global chunk slots
            base = s * 128
            lo = base + rlo
            hi = base + rhi
            j = lo // CT
            while j * CT < hi:
                a = max(lo, j * CT)
                b = min(hi, (j + 1) * CT)
                args = (a - j * CT, b - j * CT, j, etile, a - base, b - base)
                if defer:
                    pending.append(args)
                else:
                    colsum_mm(*args)
                j += 1

        def pump(n):
            # emit up to n deferred colsum matmuls into the PE queue
            for _ in range(min(n, len(pending))):
                colsum_mm(*pending.pop(0))

        def rowsum(s, codes, scr):
            # three 2x folds of the decoded bf16 values, then a 1x
            # accumulate pass over the remaining 528-col eighth
            cb = codes[:].bitcast(bf16)
            nc.vector.tensor_tensor(
                out=scr[:, 0:2112], in0=cb[:, 0:2112], in1=cb[:, 2112:4224],
                op=OP.add,
            )
            nc.vector.tensor_tensor(
                out=scr[:, 2112:3168], in0=scr[:, 0:1056],
                in1=scr[:, 1056:2112], op=OP.add,
            )
            nc.vector.tensor_tensor(
                out=scr[:, 3168:3696], in0=scr[:, 2112:2640],
                in1=scr[:, 2640:3168], op=OP.add,
            )
            nc.vector.tensor_scalar(
                out=scr[:, 3696:4224], in0=scr[:, 3168:3696],
                scalar1=1.0, scalar2=0.0, op0=OP.mult, op1=OP.add,
                accum_out=s0_t[:, s:s + 1],
            )

        # ---- main strip loop ----
        all_codes = [None] * S
        for s in range(S):
            base = s * 128  # strip start in rotated cols
            w = wpool.tile([128, SL], bf16, tag="w")
            lw = t_u2w[:, :, base:base + 128]
            scale = -(A16 * A16)
            bias = t_hsqpa[:, s:s + 1]
            # four 1024-col PSUM tiles (2 banks each, triple-buffered)
            # plus a 128-col tail tile
            for t in range(4):
                c0 = t * PT
                ps = pspool.tile([128, PT], f32, tag="ps")
                for qa, qb in ((0, 512), (512, 1024)):
                    nc.tensor.matmul(
                        ps[:, qa:qb],
                        lw,
                        t_u2[:, :, base + c0 + qa:base + c0 + qb],
                        start=True,
                        stop=True,
                        perf_mode=PM.DoubleRow,
                    )
                # w = A16 * sqrt(hsq_i - Q) = A16 * d_ij
                nc.scalar.activation(
                    out=w[:, c0:c0 + PT],
                    in_=ps[:],
                    func=AF.Sqrt,
                    scale=scale,
                    bias=bias,
                )
                # spread deferred colsum matmuls between PSUM tiles so
                # PSUM production (and thus ACT) never starves for long
                pump(3)
            pst = pstail.tile([128, 128], f32, tag="pst")
            nc.tensor.matmul(
                pst[:],
                lw,
                t_u2[:, :, base + SL - 128:base + SL],
                start=True,
                stop=True,
                perf_mode=PM.DoubleRow,
            )
            nc.scalar.activation(
                out=w[:, SL - 128:SL],
                in_=pst[:],
                func=AF.Sqrt,
                scale=scale,
                bias=bias,
            )

            # colsum pass A: one-hot matmuls over the folded accumulator
            # (rotated cols [128, 4864)); gated only on the last prefold
            # strip, so the deferred matmuls never stall the PE
            if s == max(PREFOLD) + 2:
                for j in range(NCH):
                    r_lo = max(CT * j, 128)
                    r_hi = min(CT * j + CT, 128 + FACC)
                    if r_lo >= r_hi:
                        continue
                    pending.append((r_lo - CT * j, r_hi - CT * j, j,
                                    facc, r_lo - 128, r_hi - 128))

            # direct colsums of the PREVIOUS strip: its codes are ready
            # by now, so the in-order PE queue never stalls on them
            if s - 1 >= 0 and s - 1 not in PREFOLD:
                colsums(s - 1, all_codes[s - 1][:].bitcast(bf16), 128, CW,
                        defer=True)

            codes = cpool.tile([128, SL], i16, tag="codes")
            scr = spool.tile([128, SL], bf16, tag="scr")
            all_codes[s] = codes
            if s < S - 1:
                # codes = int16(B16 - w) = bf16 bits of ~exp(-d)
                nc.vector.tensor_scalar(
                    out=codes[:], in0=w[:],
                    scalar1=-1.0, scalar2=B16, op0=OP.mult, op1=OP.add,
                )
            else:
                # last strip: chunked subtract so its PE colsums pipeline
                # with the sqrt tail; the fold-tree rowsum follows
                for c0 in (0, PT, 2 * PT, 3 * PT, SL - 128):
                    c1 = min(c0 + PT, SL)
                    nc.vector.tensor_scalar(
                        out=codes[:, c0:c1], in0=w[:, c0:c1],
                        scalar1=-1.0, scalar2=B16, op0=OP.mult, op1=OP.add,
                    )
            # self-block diagonal -> exact +0.0 (sqrt of the tiny fp8
            # rounding residual there may be NaN; the AND forces +0.0)
            nc.vector.tensor_tensor(
                out=codes[:, 0:128], in0=codes[:, 0:128],
                in1=t_dmask[:], op=OP.bitwise_and,
            )
            rowsum(s, codes, scr)
            if s in PREFOLD:
                cb = codes[:].bitcast(bf16)
                if s == min(PREFOLD):
                    nc.vector.tensor_copy(facc[:, 0:CW - 128], cb[:, 128:CW])
                else:
                    nc.vector.tensor_tensor(
                        out=facc[:, base:base + CW - 128],
                        in0=facc[:, base:base + CW - 128],
                        in1=cb[:, 128:CW], op=OP.add,
                    )

        pump(len(pending))
        # ---- colsum pass B: strip 7 direct from its codes
        colsums(S - 1, all_codes[S - 1][:].bitcast(bf16), 128, CW)

        # drain colsum accumulator: PSUM -> SBUF -> DRAM (on ACT, which
        # is idle by now -- keeps the DVE free for strip 7's fold tree)
        sb = misc.tile([NCH, CT], f32, tag="csdrain")
        nc.scalar.activation(
            out=sb[:], in_=cs_acc[:], func=AF.Copy,
        )
        # s0 is ready before the colsum drain completes: stream it first
        nc.sync.dma_start(out=h_s0[:], in_=s0_t[:])
        nc.sync.dma_start(out=h_cs[:], in_=sb[:])

    nc.finalize()
    return nc


def get_program():
    if "nc" not in _cache:
        _cache["nc"] = _build_program()
    return _cache["nc"]


def make_in_maps(x, y):
    """Host-side prep: build the per-core (column-rotated) operand arrays."""
    from concourse import mybir

    FP8 = np.dtype(mybir.dt.np(mybir.dt.float8e4))

    x = np.asarray(x, dtype=np.float32)
    y = np.asarray(y, dtype=np.float32)
    z = np.concatenate([x, y], axis=0)  # [N, D]

    u8 = (np.float32(np.sqrt(2.0)) * z).astype(FP8)        # [N, D] fp8
    uf = u8.astype(np.float32)
    hsq = np.float32(0.5) * (uf * uf).sum(axis=1, dtype=np.float32)
    hsq_hi = hsq.astype(FP8)
    hsq_lo = (hsq - hsq_hi.astype(np.float32)).astype(FP8)

    ut = np.ascontiguousarray(uf.T)  # [D, N] f32 of the fp8 values

    dmask = np.full((128, 128), -1, dtype=np.int16)
    idx = np.arange(128)
    dmask[idx, idx] = 0
    oneh = np.zeros((128, NCH * NCH), dtype=BF16)
    for j in range(NCH):
        oneh[:, NCH * j + j] = BF16(1.0)

    in_maps = []
    for c in range(NCORES):
        r0 = c * RPC
        rows = np.arange(r0, r0 + RPC)

        def rotc(a):  # rotate columns of [*, N] by -r0, crop to UCOLS
            return np.roll(a, -r0, axis=-1)[..., :UCOLS]

        utr = rotc(ut)                       # [128, UCOLS] f32
        hhr = rotc(hsq_hi[None, :])[0]       # [UCOLS] fp8
        hlr = rotc(hsq_lo[None, :])[0]       # [UCOLS] fp8

        u2 = np.zeros((65, 2, UCOLS), dtype=FP8)
        u2[0:64, 0, :] = utr[0:64].astype(FP8)
        u2[0:64, 1, :] = utr[64:128].astype(FP8)
        u2[64, 0, :] = -hhr
        u2[64, 1, :] = -hlr

        u2w = np.zeros((65, 2, S * 128), dtype=FP8)
        u2w[0:64, 0, :] = utr[0:64, :S * 128].astype(FP8)
        u2w[0:64, 1, :] = utr[64:128, :S * 128].astype(FP8)
        u2w[64, 0, :] = np.float32(1.0).astype(FP8)
        u2w[64, 1, :] = np.float32(1.0).astype(FP8)

        def pcol(vec, sel):  # [RPC] values -> [128, S] per-partition layout
            return np.ascontiguousarray(vec[sel].reshape(S, 128).T)

        hp = pcol(hsq, rows)
        consts = np.concatenate(
            [
                dmask.view(np.uint8).reshape(128, -1),
                oneh.view(np.uint8).reshape(128, -1),
                (hp * np.float32(A16 * A16)).view(np.uint8).reshape(128, -1),
            ],
            axis=1,
        )
        in_maps.append(
            {
                "u2": u2,
                "u2w": u2w,
                "consts": np.ascontiguousarray(consts),
            }
        )
    return in_maps


def finish_on_host(results, x, y):
    """Gather per-core row sums + column sums; final loss with host dp."""
    S0 = np.zeros(N, dtype=np.float64)
    for c in range(NCORES):
        r0 = c * RPC
        s0 = np.asarray(results[c]["s0"], dtype=np.float64)  # [128, S]
        cs = np.asarray(results[c]["cs"], dtype=np.float64)  # [NCH, CT]
        S0[r0:r0 + RPC] += s0[:, :S].T.reshape(-1)
        # accumulated column sums: rotated col r in [128, 4992) holds the
        # core's total colsum for global row (r0 + r) mod N
        csf = cs.reshape(-1)
        rot = np.arange(128, (S - 1) * 128 + CW)
        gidx = (r0 + rot) % N
        S0[gidx] += csf[rot]

    z = np.concatenate([np.asarray(x, np.float64), np.asarray(y, np.float64)])
    dp = np.sqrt(((z[:B] - z[B:]) ** 2).sum(axis=1))
    DP = np.concatenate([dp, dp])

    S0 /= SCALE_COMP
    tiny = float(np.finfo(np.float32).tiny)
    num = np.exp(-DP)
    loss = -np.log(num / S0 + tiny)
    return np.asarray(loss.mean(), dtype=np.float32)


def kernel(x, y):
    global LAST_RESULT
    from concourse.bass_utils import run_bass_kernel_spmd

    nc = get_program()
    in_maps = make_in_maps(x, y)
    res = run_bass_kernel_spmd(
        nc, in_maps, list(range(NCORES)), trace=PROFILE
    )
    LAST_RESULT = res
    return finish_on_host(res.results, x, y)


# revision 13
# speedup vs baseline: 1.0360x; 1.0360x over previous
"""SNN (soft-nearest-neighbor) contrastive loss on 8 Trainium2 NeuronCores.

Math
----
z = concat(x, y) in R^{8192x128};  d_ij = ||z_i - z_j||.
The row max subtracted in the reference cancels mathematically, so
    S0_i  = sum_{j != i} exp(-d_ij)          (device + host gather)
    dp_i  = d_{i, pair(i)}                   (host, O(N*D))
    loss  = mean_i( -log( exp(-dp_i)/S0_i + tiny ) )   (host, trivial)

Symmetry halving
----------------
d_ij is symmetric; each 128-row block R computes exp tiles for column
blocks R..R+32 only (self + 32 forward, cyclically).  Strip = 4224 cols.
Row sums cover the WHOLE strip (the antipodal block, offset 32, is
computed by both partners for their own rows).  Column sums cover
offsets 1..31 and are scattered on the host into the mirrored rows.

Device pipeline (one SPMD program, 8 cores, rows sharded 1024/core)
------------------------------------------------------------------
PE: fp8e4 DoubleRow matmuls with 65-row k-tiles compute
      Q = u.u^T - hsq_j      (u split into 2x64 dims; the 65th row of
k-tile 0/1 carries ones x -hsq_hi / ones x -hsq_lo, folding the hsq_j
rank-1 term into the main matmul), plus an FD-128 bf16 identity matmul
adding -LARGE on the self diagonal.
ACT: w = A16*sqrt(-Q + hsq_i) straight from PSUM via the per-partition
bias AP (bf16 out), one instruction per triple-buffered [128,1024]
PSUM tile.  ACT runs 1x/1.2GHz and is the ~40us critical engine; PE
and DVE work is split so both fit in its shadow.
DVE: Schraudolph bit trick -- one 4x-mode tensor_scalar produces
codes = int16(B16 - w) = the bf16 bit pattern of ~exp(-d)*SCALE_COMP.
Row sums: three 2x-mode fold adds then a 1x accumulate pass over the
last 528-col eighth.
Column sums, split to balance PE vs DVE:
  - prefold strips (0,4,5,6): 2x-mode adds (strip 0: 4x copy), aligned
    by rotated column, into one bf16 accumulator facc covering rotated
    cols [128, 4864); a single PE one-hot pass colsums facc at the end.
  - direct strips (1,2,3,7): PE one-hot colsum matmuls straight from
    the codes tile, emitted one strip late so the PE never stalls
    waiting on codes.
Each core gets column-ROTATED operands so every tile index is a
compile-time constant: one identical program for all 8 cores.
"""

import os
import sys
from contextlib import ExitStack

import numpy as np

_TRN_REPO = os.environ.get("TRN_RL_REPO", "/opt/trn_rl_repo")
if _TRN_REPO not in sys.path:
    sys.path.insert(0, _TRN_REPO)

import ml_dtypes

BF16 = ml_dtypes.bfloat16

B = 4096
D = 128
N = 2 * B            # 8192 rows of z
NCORES = 8
RPC = N // NCORES    # 1024 rows per core
S = RPC // 128       # 8 row-subtiles per core
CT = 512             # matmul moving tile (one PSUM bank = 512 f32)
SL = 4224            # strip length: self block + 32 forward blocks
CW = 4096            # colsum window end (blocks 1..31): [base+128, base+CW)
PT = 1024            # PSUM strip tile columns (2 banks)
UCOLS = 5120         # rotated cols touched: [0, 128*(S-1) + SL) = 5120
NCH = 10             # colsum chunks of 512 covering rotated cols [0, 5120)
FACC = (S - 2) * 128 + CW - 128   # 4736 facc cols = rotated [128, 4864)
LARGE = 7296.0       # diagonal nuke: d2 -> 7296, w -> A16*85.4 = 15772:
                     #   code = B16 - 15772 = 484 -> decoded ~4e-43 (= 0)

PREFOLD = (0, 1, 2, 3)   # strips folded into facc (0 is a copy)
WARMUP = 2

LN2 = float(np.log(2.0))
A16 = 128.0 / LN2    # bf16 exponent-code slope
B16 = 16256.0        # bf16 exponent-code offset (exact in bf16)
# with B16=16256 the Schraudolph decode averages exp(-w)*SCALE_COMP;
# the host divides all device sums by SCALE_COMP.
SCALE_COMP = 1.0406027025852233  # mean of (1+f)/2^f over f~U[0,1)

PROFILE = False
LAST_RESULT = None

_cache = {}


def _build_program():
    import concourse.tile as tile
    from concourse import bacc, mybir

    f32 = mybir.dt.float32
    bf16 = mybir.dt.bfloat16
    i16 = mybir.dt.int16
    fp8 = mybir.dt.float8e4
    AF = mybir.ActivationFunctionType
    OP = mybir.AluOpType
    PM = mybir.MatmulPerfMode

    nc = bacc.Bacc()

    u8 = mybir.dt.uint8
    h_u2 = nc.declare_dram_parameter("u2", [65, 2, UCOLS], fp8, isOutput=False)
    h_u2w = nc.declare_dram_parameter("u2w", [65, 2, S * 128], fp8, isOutput=False)
    # packed consts, one DMA: dmask | oneh | hsqpa (bytes)
    h_consts = nc.declare_dram_parameter("consts", [128, 488], u8, isOutput=False)
    h_s0 = nc.declare_dram_parameter("s0", [128, S], f32, isOutput=True)
    h_cs = nc.declare_dram_parameter("cs", [NCH, CT], f32, isOutput=True)

    # strip for subtile s covers rotated cols [s*128, s*128 + SL)
    with tile.TileContext(nc) as tc, ExitStack() as ctx:
        const = ctx.enter_context(tc.tile_pool(name="const", bufs=1))
        wpool = ctx.enter_context(tc.tile_pool(name="wbuf", bufs=3))
        cpool = ctx.enter_context(tc.tile_pool(name="codes", bufs=3))
        spool = ctx.enter_context(tc.tile_pool(name="scr", bufs=2))
        pspool = ctx.enter_context(tc.tile_pool(name="ps", bufs=3, space="PSUM"))
        pstail = ctx.enter_context(tc.tile_pool(name="pst", bufs=1, space="PSUM"))
        cspool = ctx.enter_context(tc.tile_pool(name="cps", bufs=1, space="PSUM"))
        misc = ctx.enter_context(tc.tile_pool(name="misc", bufs=1))

        # DMA priority: first matmul's operands first, then packed consts,
        # then the rest of u2 in chunks
        t_u2w = const.tile([65, 2, S * 128], fp8)
        nc.sync.dma_start(out=t_u2w[:], in_=h_u2w[:])
        t_u2 = const.tile([65, 2, UCOLS], fp8)
        edges = [0, 512, 1024, 2048, 3072, 4096, 5120]
        for a, b in zip(edges[:2], edges[1:3]):
            nc.sync.dma_start(out=t_u2[:, :, a:b], in_=h_u2[:, :, a:b])

        t_consts = const.tile([128, 488], u8)
        nc.sync.dma_start(out=t_consts[:], in_=h_consts[:])
        t_dmask = t_consts[:, 0:256].bitcast(i16)
        t_oneh = t_consts[:, 256:456].bitcast(bf16)
        t_hsqpa = t_consts[:, 456:488].bitcast(f32)

        for a, b in zip(edges[2:-1], edges[3:]):
            nc.sync.dma_start(out=t_u2[:, :, a:b], in_=h_u2[:, :, a:b])

        t_zero10 = const.tile([128, NCH], bf16)
        nc.vector.memset(t_zero10[:], 0.0)
        t_z512 = const.tile([128, CT], bf16)
        nc.vector.memset(t_z512[:], 0.0)

        s0_t = const.tile([128, S], f32)
        facc = const.tile([128, FACC], bf16)
        # zero the tail of facc not covered by the strip-0 copy
        nc.gpsimd.memset(facc[:, CW - 128:FACC], 0.0)

        # dummy tiny sqrt: triggers the sqrt ACT_TABLE_LOAD during the
        # input DMA instead of lazily before the first real sqrt
        t_dummy = const.tile([128, 1], f32)
        nc.vector.memset(t_dummy[:], 1.0)
        dummyout = const.tile([128, 1], bf16)
        nc.scalar.activation(
            out=dummyout[:], in_=t_dummy[:], func=AF.Sqrt, scale=1.0
        )

        # single resident colsum accumulator [NCH, 512]
        cs_acc = cspool.tile([NCH, CT], f32, tag="cs", name="cs_acc")

        # zero the colsum accumulator (matmul with zero weights); also
        # keeps the PE busy during the input DMA
        for rep in range(WARMUP):
            nc.tensor.matmul(
                cs_acc[:], t_zero10[:], t_z512[:],
                start=(rep == 0), stop=False, skip_group_check=True,
            )

        pending = []

        def colsum_mm(cs_a, cs_b, j, etile, e_a, e_b):
            nc.tensor.matmul(
                cs_acc[:, cs_a:cs_b],
                t_oneh[:, NCH * j:NCH * (j + 1)],
                etile[:, e_a:e_b],
                start=False,
                stop=False,
                skip_group_check=True,
            )

        def colsums(s, etile, rlo, rhi, defer=False):
            # accumulate per-column sums of etile cols [rlo, rhi) (strip-
            # local) into cs_acc at their global chunk slots
            base = s * 128
            lo = base + rlo
            hi = base + rhi
            j = lo // CT
            while j * CT < hi:
                a = max(lo, j * CT)
                b = min(hi, (j + 1) * CT)
                args = (a - j * CT, b - j * CT, j, etile, a - base, b - base)
                if defer:
                    pending.append(args)
                else:
                    colsum_mm(*args)
                j += 1

        def pump(n):
            # emit up to n deferred colsum matmuls into the PE queue
            for _ in range(min(n, len(pending))):
                colsum_mm(*pending.pop(0))

        def rowsum(s, codes, scr):
            # three 2x folds of the decoded bf16 values, then a 1x
            # accumulate pass over the remaining 528-col eighth
            cb = codes[:].bitcast(bf16)
            nc.vector.tensor_tensor(
                out=scr[:, 0:2112], in0=cb[:, 0:2112], in1=cb[:, 2112:4224],
                op=OP.add,
            )
            nc.vector.tensor_tensor(
                out=scr[:, 2112:3168], in0=scr[:, 0:1056],
                in1=scr[:, 1056:2112], op=OP.add,
            )
            nc.vector.tensor_tensor(
                out=scr[:, 3168:3696], in0=scr[:, 2112:2640],
                in1=scr[:, 2640:3168], op=OP.add,
            )
            nc.vector.tensor_scalar(
                out=scr[:, 3696:4224], in0=scr[:, 3168:3696],
                scalar1=1.0, scalar2=0.0, op0=OP.mult, op1=OP.add,
                accum_out=s0_t[:, s:s + 1],
            )

        # ---- main strip loop ----
        all_codes = [None] * S
        for s in range(S):
            base = s * 128  # strip start in rotated cols
            w = wpool.tile([128, SL], bf16, tag="w")
            lw = t_u2w[:, :, base:base + 128]
            scale = -(A16 * A16)
            bias = t_hsqpa[:, s:s + 1]
            # four 1024-col PSUM tiles (2 banks each, triple-buffered)
            # plus a 128-col tail tile
            for t in range(4):
                c0 = t * PT
                ps = pspool.tile([128, PT], f32, tag="ps")
                for qa, qb in ((0, 512), (512, 1024)):
                    nc.tensor.matmul(
                        ps[:, qa:qb],
                        lw,
                        t_u2[:, :, base + c0 + qa:base + c0 + qb],
                        start=True,
                        stop=True,
                        perf_mode=PM.DoubleRow,
                    )
                # w = A16 * sqrt(hsq_i - Q) = A16 * d_ij
                nc.scalar.activation(
                    out=w[:, c0:c0 + PT],
                    in_=ps[:],
                    func=AF.Sqrt,
                    scale=scale,
                    bias=bias,
                )
                # spread deferred colsum matmuls between PSUM tiles so
                # PSUM production (and thus ACT) never starves for long
                pump(3)
            pst = pstail.tile([128, 128], f32, tag="pst")
            nc.tensor.matmul(
                pst[:],
                lw,
                t_u2[:, :, base + SL - 128:base + SL],
                start=True,
                stop=True,
                perf_mode=PM.DoubleRow,
            )
            nc.scalar.activation(
                out=w[:, SL - 128:SL],
                in_=pst[:],
                func=AF.Sqrt,
                scale=scale,
                bias=bias,
            )

            # colsum pass A: one-hot matmuls over the folded accumulator
            # (rotated cols [128, 4864)); gated only on the last prefold
            # strip, so the deferred matmuls never stall the PE
            if s == max(PREFOLD) + 2:
                for j in range(NCH):
                    r_lo = max(CT * j, 128)
                    r_hi = min(CT * j + CT, 128 + FACC)
                    if r_lo >= r_hi:
                        continue
                    pending.append((r_lo - CT * j, r_hi - CT * j, j,
                                    facc, r_lo - 128, r_hi - 128))

            # direct colsums of the PREVIOUS strip: its codes are ready
            # by now, so the in-order PE queue never stalls on them
            if s - 1 >= 0 and s - 1 not in PREFOLD:
                colsums(s - 1, all_codes[s - 1][:].bitcast(bf16), 128, CW,
                        defer=True)

            codes = cpool.tile([128, SL], i16, tag="codes")
            scr = spool.tile([128, SL], bf16, tag="scr")
            all_codes[s] = codes
            if s < S - 1:
                # codes = int16(B16 - w) = bf16 bits of ~exp(-d)
                nc.vector.tensor_scalar(
                    out=codes[:], in0=w[:],
                    scalar1=-1.0, scalar2=B16, op0=OP.mult, op1=OP.add,
                )
            else:
                # last strip: chunked subtract so its PE colsums pipeline
                # with the sqrt tail; the fold-tree rowsum follows
                for c0 in (0, PT, 2 * PT, 3 * PT, SL - 128):
                    c1 = min(c0 + PT, SL)
                    nc.vector.tensor_scalar(
                        out=codes[:, c0:c1], in0=w[:, c0:c1],
                        scalar1=-1.0, scalar2=B16, op0=OP.mult, op1=OP.add,
                    )
            # self-block diagonal -> exact +0.0 (sqrt of the tiny fp8
            # rounding residual there may be NaN; the AND forces +0.0)
            nc.vector.tensor_tensor(
                out=codes[:, 0:128], in0=codes[:, 0:128],
                in1=t_dmask[:], op=OP.bitwise_and,
            )
            rowsum(s, codes, scr)
            if s in PREFOLD:
                cb = codes[:].bitcast(bf16)
                if s == min(PREFOLD):
                    nc.vector.tensor_copy(facc[:, 0:CW - 128], cb[:, 128:CW])
                else:
                    nc.vector.tensor_tensor(
                        out=facc[:, base:base + CW - 128],
                        in0=facc[:, base:base + CW - 128],
                        in1=cb[:, 128:CW], op=OP.add,
                    )

        pump(len(pending))
        # ---- colsum pass B: strip 7 direct from its codes
        colsums(S - 1, all_codes[S - 1][:].bitcast(bf16), 128, CW)

        # drain colsum accumulator: PSUM -> SBUF -> DRAM
        sb = misc.tile([NCH, CT], f32, tag="csdrain")
        nc.vector.tensor_copy(sb[:], cs_acc[:])
        # s0 is ready before the colsum drain completes: stream it first
        nc.sync.dma_start(out=h_s0[:], in_=s0_t[:])
        nc.sync.dma_start(out=h_cs[:], in_=sb[:])

    nc.finalize()
    return nc


def get_program():
    if "nc" not in _cache:
        _cache["nc"] = _build_program()
    return _cache["nc"]


def make_in_maps(x, y):
    """Host-side prep: build the per-core (column-rotated) operand arrays."""
    from concourse import mybir

    FP8 = np.dtype(mybir.dt.np(mybir.dt.float8e4))

    x = np.asarray(x, dtype=np.float32)
    y = np.asarray(y, dtype=np.float32)
    z = np.concatenate([x, y], axis=0)  # [N, D]

    u8 = (np.float32(np.sqrt(2.0)) * z).astype(FP8)        # [N, D] fp8
    uf = u8.astype(np.float32)
    hsq = np.float32(0.5) * (uf * uf).sum(axis=1, dtype=np.float32)
    hsq_hi = hsq.astype(FP8)
    hsq_lo = (hsq - hsq_hi.astype(np.float32)).astype(FP8)

    ut = np.ascontiguousarray(uf.T)  # [D, N] f32 of the fp8 values

    dmask = np.full((128, 128), -1, dtype=np.int16)
    idx = np.arange(128)
    dmask[idx, idx] = 0
    oneh = np.zeros((128, NCH * NCH), dtype=BF16)
    for j in range(NCH):
        oneh[:, NCH * j + j] = BF16(1.0)

    in_maps = []
    for c in range(NCORES):
        r0 = c * RPC
        rows = np.arange(r0, r0 + RPC)

        def rotc(a):  # rotate columns of [*, N] by -r0, crop to UCOLS
            return np.roll(a, -r0, axis=-1)[..., :UCOLS]

        utr = rotc(ut)                       # [128, UCOLS] f32
        hhr = rotc(hsq_hi[None, :])[0]       # [UCOLS] fp8
        hlr = rotc(hsq_lo[None, :])[0]       # [UCOLS] fp8

        u2 = np.zeros((65, 2, UCOLS), dtype=FP8)
        u2[0:64, 0, :] = utr[0:64].astype(FP8)
        u2[0:64, 1, :] = utr[64:128].astype(FP8)
        u2[64, 0, :] = -hhr
        u2[64, 1, :] = -hlr

        u2w = np.zeros((65, 2, S * 128), dtype=FP8)
        u2w[0:64, 0, :] = utr[0:64, :S * 128].astype(FP8)
        u2w[0:64, 1, :] = utr[64:128, :S * 128].astype(FP8)
        u2w[64, 0, :] = np.float32(1.0).astype(FP8)
        u2w[64, 1, :] = np.float32(1.0).astype(FP8)

        def pcol(vec, sel):  # [RPC] values -> [128, S] per-partition layout
            return np.ascontiguousarray(vec[sel].reshape(S, 128).T)

        hp = pcol(hsq, rows)
        consts = np.concatenate(
            [
                dmask.view(np.uint8).reshape(128, -1),
                oneh.view(np.uint8).reshape(128, -1),
                (hp * np.float32(A16 * A16)).view(np.uint8).reshape(128, -1),
            ],
            axis=1,
        )
        in_maps.append(
            {
                "u2": u2,
                "u2w": u2w,
                "consts": np.ascontiguousarray(consts),
            }
        )
    return in_maps


def finish_on_host(results, x, y):
    """Gather per-core row sums + column sums; final loss with host dp."""
    S0 = np.zeros(N, dtype=np.float64)
    for c in range(NCORES):
        r0 = c * RPC
        s0 = np.asarray(results[c]["s0"], dtype=np.float64)  # [128, S]
        cs = np.asarray(results[c]["cs"], dtype=np.float64)  # [NCH, CT]
        S0[r0:r0 + RPC] += s0[:, :S].T.reshape(-1)
        # accumulated column sums: rotated col r in [128, 4992) holds the
        # core's total colsum for global row (r0 + r) mod N
        csf = cs.reshape(-1)
        rot = np.arange(128, (S - 1) * 128 + CW)
        gidx = (r0 + rot) % N
        S0[gidx] += csf[rot]

    z = np.concatenate([np.asarray(x, np.float64), np.asarray(y, np.float64)])
    dp = np.sqrt(((z[:B] - z[B:]) ** 2).sum(axis=1))
    DP = np.concatenate([dp, dp])

    S0 /= SCALE_COMP
    tiny = float(np.finfo(np.float32).tiny)
    num = np.exp(-DP)
    loss = -np.log(num / S0 + tiny)
    return np.asarray(loss.mean(), dtype=np.float32)


def kernel(x, y):
    global LAST_RESULT
    from concourse.bass_utils import run_bass_kernel_spmd

    nc = get_program()
    in_maps = make_in_maps(x, y)
    res = run_bass_kernel_spmd(
        nc, in_maps, list(range(NCORES)), trace=PROFILE
    )
    LAST_RESULT = res
    return finish_on_host(res.results, x, y)
